# revision 35
# baseline (speedup 1.0000x reference)
"""
Trainium2 Bass kernel for nn_MetaAttention.

Computation (per batch b):
    rowsum[h,i]     = sum_j m[b,h,i,j]
    aggregated[i,j] = sum_h rowsum[h,i] * m[b,h,i,j]
    out[b]          = softmax(aggregated.flatten()).reshape(N, N)

Sharding: pure data parallel over B=16 across 8 cores (2 batches/core).

Memory-regime kernel, ~64 MB HBM traffic/core. Profiling facts that
drive the design:
  (1) SDMA per-engine rate collapses to ~16 GB/s for 112-partition SBUF
      tiles (784 = 112*7) but hits the full ~27 GB/s/engine (350-400
      GB/s/core over the two HWDGE rings) for 128-partition tiles. So
      partition p holds rows 6p..6p+5 ("slots" 0-5) -> [128, 6, 784]
      main loads, 18816 B contiguous per partition, alternating
      sync/scalar HWDGE queues; leftover rows 768..783 ride as a
      [16, 784] "extra" (slot 6) on partitions 64..79 whose dead lanes
      are never read.
  (2) every engine has large per-op fixed costs (ACT ~0.5us, DVE
      ~0.6us, GpSimd ~1.3us, plus ~0.3us per inner-dim AP restart) and
      fp32 matmuls run as LOW+HIGH pass pairs, so per-head op count is
      minimized and split so no engine exceeds its share:
      - rowsums: ONE DVE multi-slot reduce (slots 0-3) + 3 ACT
        activation+accum ops (slots 4, 5 and the extras)
      - ONE DVE broadcast-multiply builds the diag matrices of all 7
        slots at once: dg[p,s,i] = ident[p,i] * rs[p,s]
      - PE accumulates j in [0,512) of slots 0-5 over heads (diag
        matmul, one PSUM bank each) and the extras full-width via a
        K=16 contraction over its 16 live partitions (2 banks)
      - the j in [512,784) tail of slots 0-5 is ONE GpSimd
        broadcast-mult + ONE GpSimd add into the SBUF agg per head

Softmax: batch 0 merges PSUM->SBUF with ACT copies at its last head
(frees all 8 PSUM banks for batch 1 fast) and is staged into batch 1's
head loop at 1-2 ops per head (gpsimd partition_all_reduce for the
cross-partition max/sum) so the load stream never pauses. Batch 1 (the
tail) skips merging: exp reads the PSUM accumulators directly with the
bias subtract fused; the cross-partition max uses gpsimd all_reduce,
the sum a PE ones-matmul into a retired PSUM bank; stores go out on
the then-idle HWDGE queues.

NOTE: tensor_tensor_reduce (custom DVE uop) wedges the exec unit on
real HW - do not use it.
"""

import numpy as np

B, H, N = 16, 12, 784
NCORES = 8
BPC = B // NCORES          # batches per core
P = 128                    # partitions
S = 6                      # main row-slots per partition (rows 0..767)
NS = 7                     # total slots incl. the extras slot
NME = P * S                # 768 main rows
XPB = 64                   # extra rows 768..783 live on partitions 64..79
XPE = XPB + 16             # (base 64: legal for engine APs and PE lhsT)
NPE = 6                    # all six main slots on PE, 512-wide
JP = 512                   # PE accumulates j in [0,JP); GpSimd the tail
JT = N - JP                # 272
JSPLITS = [(0, 512), (512, 272)]  # fp32 matmul free-dim <= 512, bank aligned
MHBUFS = 6

LAST_RESULT = None  # BassKernelResults of the most recent kernel() call


def build_program():
    import concourse.bacc as bacc
    import concourse.tile as tile
    from concourse import mybir
    from concourse import bass_isa

    f32 = mybir.dt.float32
    AX = mybir.AxisListType.X
    ADD = mybir.AluOpType.add
    MULT = mybir.AluOpType.mult
    MAXOP = mybir.AluOpType.max
    COPYF = mybir.ActivationFunctionType.Copy
    EXPF = mybir.ActivationFunctionType.Exp

    nc = bacc.Bacc("TRN2")
    x = nc.dram_tensor("x", [BPC, H, N, N], f32, kind="ExternalInput")
    ident = nc.dram_tensor("ident", [P, P], f32, kind="ExternalInput")
    y = nc.dram_tensor("y", [BPC, N, N], f32, kind="ExternalOutput")

    with tile.TileContext(nc) as tc:
        with (
            tc.tile_pool(name="mh", bufs=MHBUFS) as mh_pool,
            tc.tile_pool(name="agg", bufs=2) as agg_pool,
            tc.tile_pool(name="acc", bufs=NPE, space="PSUM") as acc_pool,
            tc.tile_pool(name="acc6", bufs=1, space="PSUM") as acc6_pool,
            tc.tile_pool(name="sct", bufs=2) as sct_pool,
            tc.tile_pool(name="dgp", bufs=3) as dg_pool,
            tc.tile_pool(name="scr", bufs=1) as scr_pool,
            tc.tile_pool(name="sc2", bufs=3) as sc2_pool,
            tc.tile_pool(name="small", bufs=6) as small_pool,
            tc.tile_pool(name="consts", bufs=1) as const_pool,
        ):
            ident_sb = const_pool.tile([P, P], f32)
            nc.sync.dma_start(out=ident_sb, in_=ident[:, :])
            ones_sb = const_pool.tile([P, P], f32)
            nc.vector.memset(ones_sb, 1.0)
            idst = const_pool.tile([P, NS, P], f32)
            for a in range(NS):
                nc.scalar.dma_start(out=idst[:, a, :], in_=ident[:, :])

            state = {}

            def emit_head(b, h):
                st = state[b]
                agg, accs = st["agg"], st["accs"]
                gi = b * H + h
                qm, qx = (nc.sync, nc.scalar) if gi % 2 == 0 else (nc.scalar, nc.sync)
                mh = mh_pool.tile([P, NS, N], f32, tag="mh")
                qm.dma_start(
                    out=mh[:, 0:S, :],
                    in_=x[b, h, 0:NME, :].rearrange("(p t) j -> p t j", p=P),
                )
                qx.dma_start(out=mh[XPB:XPE, S, :], in_=x[b, h, NME:N, :])

                rs = small_pool.tile([P, 8], f32, tag="rs")
                nc.vector.memset(rs[:, 6:7], 0.0)
                nc.vector.tensor_reduce(
                    out=rs[:, 0:4], in_=mh[:, 0:4, :], axis=AX, op=ADD
                )
                for s in (4, 5):
                    scr = scr_pool.tile([P, N], f32, tag="scr")
                    nc.scalar.activation(
                        out=scr, in_=mh[:, s, :], func=COPYF, bias=0.0,
                        scale=1.0, accum_out=rs[:, s : s + 1],
                    )
                scr6 = scr_pool.tile([P, N], f32, tag="scr")
                nc.scalar.activation(
                    out=scr6[XPB:XPE, :], in_=mh[XPB:XPE, S, :], func=COPYF,
                    bias=0.0, scale=1.0, accum_out=rs[XPB:XPE, 6:7],
                )
                # diag matrices for all 7 slots in one DVE op
                dg = dg_pool.tile([P, NS, P], f32, tag="dg")
                rs_bc = rs[:, 0:NS].unsqueeze(2).broadcast_to([P, NS, P])
                nc.vector.tensor_tensor(out=dg, in0=idst, in1=rs_bc, op=MULT)
                last = h == H - 1
                acc6 = st["acc6"]
                # PE: j in [0,512) of slots 0-5 (one bank each) and the
                # extras full-width via a K=16 contraction over its 16
                # live partitions (dead lanes never read)
                for s in range(NPE):
                    nc.tensor.matmul(
                        accs[s][:, 0:JP],
                        lhsT=dg[:, s, :],
                        rhs=mh[:, s, 0:JP],
                        start=(h == 0),
                        stop=last,
                    )
                for j0, jn in ((0, JP), (JP, JT)):
                    nc.tensor.matmul(
                        acc6[:, j0 : j0 + jn],
                        lhsT=dg[XPB:XPE, S, :],
                        rhs=mh[XPB:XPE, S, j0 : j0 + jn],
                        start=(h == 0),
                        stop=last,
                    )
                # GpSimd: the j in [512,784) tail of slots 0-5 in two ops
                rs_bct = rs[:, 0:S].unsqueeze(2).broadcast_to([P, S, JT])
                if h == 0:
                    nc.gpsimd.tensor_tensor(
                        out=agg[:, 0:S, JP:N], in0=mh[:, 0:S, JP:N],
                        in1=rs_bct, op=MULT,
                    )
                elif last:
                    # final head of either batch: scale on ACT per slot
                    # (overlaps the PE matmuls) so the GpSimd queue only
                    # carries the one add at the batch handoff / tail
                    sct = sct_pool.tile([P, S, JT], f32, tag="sct")
                    for s in range(S):
                        nc.scalar.activation(
                            out=sct[:, s, :], in_=mh[:, s, JP:N], func=COPYF,
                            bias=0.0, scale=rs[:, s : s + 1],
                        )
                    nc.gpsimd.tensor_tensor(
                        out=agg[:, 0:S, JP:N], in0=sct, in1=agg[:, 0:S, JP:N],
                        op=ADD,
                    )
                else:
                    sct = sct_pool.tile([P, S, JT], f32, tag="sct")
                    nc.gpsimd.tensor_tensor(
                        out=sct, in0=mh[:, 0:S, JP:N], in1=rs_bct, op=MULT
                    )
                    nc.gpsimd.tensor_tensor(
                        out=agg[:, 0:S, JP:N], in0=sct, in1=agg[:, 0:S, JP:N],
                        op=ADD,
                    )
                if last and b == 0:
                    # merge PSUM->SBUF now so batch 1 can reuse the banks
                    for s in range(NPE):
                        nc.scalar.activation(
                            out=agg[:, s, 0:JP], in_=accs[s][:, 0:JP],
                            func=COPYF, bias=0.0, scale=1.0,
                        )
                    nc.scalar.activation(
                        out=agg[:, S, :], in_=acc6[:, 0:N],
                        func=COPYF, bias=0.0, scale=1.0,
                    )

            def emit_softmax_b0(stg):
                """Full-SBUF softmax for batch 0, staged into batch 1's
                head loop at 1-2 ops per head so batch 1's per-head
                RS->dg->PE chain is never delayed."""
                st = state[0]
                agg, maxs, sums = st["agg"], st["maxs"], st["sums"]
                if stg in (0, 1, 2):
                    for s in (stg * 2, stg * 2 + 1):
                        nc.vector.tensor_reduce(
                            out=maxs[:, s : s + 1], in_=agg[:, s, :],
                            axis=AX, op=MAXOP,
                        )
                    if stg == 2:
                        nc.vector.tensor_reduce(
                            out=maxs[:, 6:7], in_=agg[:, S, :],
                            axis=AX, op=MAXOP,
                        )
                elif stg == 3:
                    m1 = small_pool.tile([P, 1], f32, tag="m1")
                    nc.vector.tensor_reduce(
                        out=m1, in_=maxs[:, 0:NS], axis=AX, op=MAXOP
                    )
                    gmax = small_pool.tile([P, 1], f32, tag="gmax")
                    nc.gpsimd.partition_all_reduce(
                        gmax, m1, P, bass_isa.ReduceOp.max
                    )
                    negmax = small_pool.tile([P, 1], f32, tag="negmax")
                    nc.scalar.mul(out=negmax, in_=gmax, mul=-1.0)
                    st["negmax"] = negmax
                elif stg in (4, 5, 6):
                    negmax = st["negmax"]
                    for s in (stg * 2 - 8, stg * 2 - 7):
                        nc.scalar.activation(
                            out=agg[:, s, :], in_=agg[:, s, :], func=EXPF,
                            bias=negmax, scale=1.0,
                            accum_out=sums[:, s : s + 1],
                        )
                    if stg == 6:
                        nc.scalar.activation(
                            out=agg[:, S, :], in_=agg[:, S, :],
                            func=EXPF, bias=negmax, scale=1.0,
                            accum_out=sums[:, 6:7],
                        )
                elif stg == 7:
                    s1 = small_pool.tile([P, 1], f32, tag="s1")
                    nc.vector.tensor_reduce(
                        out=s1, in_=sums[:, 0:NS], axis=AX, op=ADD
                    )
                    gsum = small_pool.tile([P, 1], f32, tag="gsum")
                    nc.gpsimd.partition_all_reduce(
                        gsum, s1, P, bass_isa.ReduceOp.add
                    )
                    rinv = small_pool.tile([P, 1], f32, tag="rinv")
                    nc.vector.reciprocal(out=rinv, in_=gsum)
                    st["rinv"] = rinv
                elif stg in (8, 9, 10):
                    s0 = (stg - 8) * 2
                    rinv = st["rinv"]
                    nc.scalar.activation(
                        out=agg[:, s0, :], in_=agg[:, s0, :], func=COPYF,
                        bias=0.0, scale=rinv,
                    )
                    nc.vector.tensor_scalar_mul(
                        out=agg[:, s0 + 1, :], in0=agg[:, s0 + 1, :],
                        scalar1=rinv,
                    )
                    dst = y[0, 0:NME, :].rearrange("(p t) j -> p t j", p=P)
                    nc.gpsimd.dma_start(
                        out=dst[:, s0 : s0 + 2, :], in_=agg[:, s0 : s0 + 2, :]
                    )
                else:  # stage 11: extras
                    rinv = st["rinv"]
                    nc.scalar.activation(
                        out=agg[XPB:XPE, S, :], in_=agg[XPB:XPE, S, :],
                        func=COPYF, bias=0.0, scale=rinv[XPB:XPE, :],
                    )
                    nc.gpsimd.dma_start(
                        out=y[0, NME:N, :], in_=agg[XPB:XPE, S, :]
                    )

            def emit_tail_b1():
                """Batch 1 softmax: maxes off PSUM mains + SBUF tails,
                exp reads PSUM directly (no merge), gpsimd all_reduce
                for the cross-partition max, a PE ones-matmul into a
                retired PSUM bank for the sum, stores on the idle HWDGE
                queues."""
                st = state[1]
                agg, maxs, sums, accs, acc6 = (
                    st["agg"], st["maxs"], st["sums"], st["accs"], st["acc6"],
                )
                for s in range(NPE):
                    nc.vector.tensor_reduce(
                        out=maxs[:, s : s + 1], in_=accs[s][:, 0:JP],
                        axis=AX, op=MAXOP,
                    )
                nc.vector.tensor_reduce(
                    out=maxs[:, 6:7], in_=acc6[:, 0:N], axis=AX, op=MAXOP
                )
                mtail = small_pool.tile([P, 8], f32, tag="mtail")
                nc.vector.memset(mtail, -1e30)
                nc.vector.tensor_reduce(
                    out=mtail[:, 0:S], in_=agg[:, 0:S, JP:N], axis=AX,
                    op=MAXOP,
                )
                comb = small_pool.tile([P, 8], f32, tag="comb")
                nc.vector.tensor_tensor(out=comb, in0=maxs, in1=mtail, op=MAXOP)
                m1 = small_pool.tile([P, 1], f32, tag="m1")
                nc.vector.tensor_reduce(
                    out=m1, in_=comb[:, 0:NS], axis=AX, op=MAXOP
                )
                gmax = small_pool.tile([P, 1], f32, tag="gmax")
                nc.gpsimd.partition_all_reduce(
                    gmax, m1, P, bass_isa.ReduceOp.max
                )
                negmax = small_pool.tile([P, 1], f32, tag="negmax")
                nc.scalar.mul(out=negmax, in_=gmax, mul=-1.0)
                for s in range(NPE):
                    nc.scalar.activation(
                        out=agg[:, s, 0:JP], in_=accs[s][:, 0:JP], func=EXPF,
                        bias=negmax, scale=1.0, accum_out=sums[:, s : s + 1],
                    )
                nc.scalar.activation(
                    out=agg[:, S, :], in_=acc6[:, 0:N], func=EXPF,
                    bias=negmax, scale=1.0, accum_out=sums[:, 6:7],
                )
                nc.scalar.activation(
                    out=agg[:, 0:S, JP:N], in_=agg[:, 0:S, JP:N], func=EXPF,
                    bias=negmax, scale=1.0, accum_out=sums[:, 7:8],
                )
                s1 = small_pool.tile([P, 1], f32, tag="s1")
                nc.vector.tensor_reduce(out=s1, in_=sums, axis=AX, op=ADD)
                # cross-partition sum + broadcast via ones-matmul into a
                # PSUM bank whose exp already retired
                sps = acc_pool.tile([P, JP], f32, tag="acc", name="sps1")
                nc.tensor.matmul(
                    sps[:, 0:1], lhsT=ones_sb, rhs=s1, start=True, stop=True
                )
                rinv = small_pool.tile([P, 1], f32, tag="rinv")
                nc.vector.reciprocal(out=rinv, in_=sps[:, 0:1])
                dst = y[1, 0:NME, :].rearrange("(p t) j -> p t j", p=P)
                for pi in range(3):
                    s0 = pi * 2
                    nc.scalar.activation(
                        out=agg[:, s0, :], in_=agg[:, s0, :], func=COPYF,
                        bias=0.0, scale=rinv,
                    )
                    nc.vector.tensor_scalar_mul(
                        out=agg[:, s0 + 1, :], in0=agg[:, s0 + 1, :],
                        scalar1=rinv,
                    )
                    eng = nc.sync if pi % 2 == 0 else nc.scalar
                    eng.dma_start(
                        out=dst[:, s0 : s0 + 2, :], in_=agg[:, s0 : s0 + 2, :]
                    )
                nc.scalar.activation(
                    out=agg[XPB:XPE, S, :], in_=agg[XPB:XPE, S, :],
                    func=COPYF, bias=0.0, scale=rinv[XPB:XPE, :],
                )
                nc.sync.dma_start(out=y[1, NME:N, :], in_=agg[XPB:XPE, S, :])

            STAGE_AT = {h: [h] for h in range(12)}

            for b in range(BPC):
                agg = agg_pool.tile([P, NS, N], f32, tag="agg")
                maxs = small_pool.tile([P, 8], f32, tag="maxs")
                sums = small_pool.tile([P, 8], f32, tag="sums")
                nc.vector.memset(maxs, -1e30)
                nc.vector.memset(sums, 0.0)
                accs = [
                    acc_pool.tile([P, JP], f32, tag="acc", name=f"acc_{b}_{s}")
                    for s in range(NPE)
                ]
                acc6 = acc6_pool.tile([P, 1024], f32, tag="acc6", name=f"acc6_{b}")
                state[b] = dict(
                    agg=agg, maxs=maxs, sums=sums, accs=accs, acc6=acc6
                )
                for h in range(H):
                    emit_head(b, h)
                    if b == 1:
                        for stg in STAGE_AT.get(h, []):
                            emit_softmax_b0(stg)
            emit_tail_b1()

    nc.finalize()
    return nc


def kernel(mha_masks) -> np.ndarray:
    global LAST_RESULT
    from concourse.bass_utils import run_bass_kernel_spmd

    xfull = np.ascontiguousarray(np.asarray(mha_masks, dtype=np.float32))
    assert xfull.shape == (B, H, N, N), xfull.shape

    nc = build_program()
    ident = np.eye(P, dtype=np.float32)
    in_maps = [
        {"x": xfull[i * BPC : (i + 1) * BPC], "ident": ident}
        for i in range(NCORES)
    ]
    import os

    kw = {}
    if os.environ.get("KERNEL_TRACE_DIR"):
        kw = dict(trace=True, tmpdir=os.environ["KERNEL_TRACE_DIR"])
    res = run_bass_kernel_spmd(nc, in_maps, core_ids=list(range(NCORES)), **kw)
    LAST_RESULT = res
    out = np.concatenate(
        [np.asarray(r["y"], dtype=np.float32) for r in res.results], axis=0
    )
    return out
